# revision 62
# baseline (speedup 1.0000x reference)
"""A3TGCN (GCN + GRU-with-zero-state + attention) on 8 TRN2 NeuronCores.

Math (reference collapses because H0 == 0 every period):
    y   = A_norm @ X            # one SpMM, X = x reshaped [N, F*T] (192 cols)
    Zc_p = sigmoid(y_p @ Uz + cz)          # == 1 - Z_p  (weights negated)
    Ht_p = tanh   (y_p @ Uh + ch)
    Hacc = sum_p probs_p * Zc_p * Ht_p
    out  = relu(Hacc) @ W_out.T + b_out    # [N, 12]
where Uz = -(W_z @ lin_Wz[:, :O].T), etc. (GCN weight commutes past A_norm;
R gate multiplies H0 = 0 so it vanishes.)

Sharding: nodes partitioned by destination across 8 cores (graph parallel),
X replicated, no collectives. Edges are sorted by (core, 128-node dst group,
src half), padded to a fixed per-(group,half) tile count so all 8 cores run
one SPMD instruction stream. Per core the SpMM is: dma_gather of bf16 node
rows (512 B each) + PE matmul against a host-built norm-scaled one-hot
(segment sum in PSUM). Dense GRU runs with O on partitions so the gate
biases ride the ScalarEngine's per-partition bias.
"""

import os
import sys

sys.path.insert(0, "/opt/trn_rl_repo")

import numpy as np
import ml_dtypes

BF16 = ml_dtypes.bfloat16

N, F, T, O, E = 50000, 16, 12, 128, 800000
NCORES = 8
NPC = N // NCORES  # 6250 nodes per core
HALF = N // 2  # 25000 (int16 gather index limit)
G = (NPC + 127) // 128  # 49 real dst groups of 128 nodes
GPC = 4  # groups per gather call
GP = ((G + GPC - 1) // GPC) * GPC  # 52 padded group slots
NCALLS = GP // GPC  # 13

LAST = None  # BassKernelResults of the most recent run (test.py reads this)


def _softmax(a):
    a = np.asarray(a, np.float32)
    e = np.exp(a - a.max())
    return e / e.sum()


def _wrap16(v):
    """Flat int16 index vector -> dma_gather layout [128, n//16].

    Index j lives at [j%16, j//16]; the 16-partition block is replicated to
    all 8 Q7-core partition windows because queue q's rx/tx cores (2q, 2q+1)
    each read their own 16-partition window (channels=(queue+1)*2*16).
    """
    n = v.shape[-1]
    w = v.reshape(*v.shape[:-1], n // 16, 16).swapaxes(-1, -2)
    return np.ascontiguousarray(np.concatenate([w] * 8, axis=-2))


def _build_graph(T0, T1, t0s, t1s, debug=False):
    """t0s/t1s: per-group-slot tile counts (max over the 8 cores), so every
    core runs the same instruction stream but pad work tracks the per-group
    max rather than the global max."""
    import concourse.bacc as bacc
    from concourse import mybir, tile

    dt = mybir.dt
    AF = mybir.ActivationFunctionType
    ALU = mybir.AluOpType
    TT = T0 + T1

    nc = bacc.Bacc(None, target_bir_lowering=False, num_swdge_queues=4)

    xlo = nc.declare_dram_parameter("xlo", [HALF, 256], dt.bfloat16, isOutput=False)
    xhi = nc.declare_dram_parameter("xhi", [HALF, 256], dt.bfloat16, isOutput=False)
    ilo_d = nc.declare_dram_parameter(
        "ilo", [GP, 128, T0 * 128 // 16], dt.int16, isOutput=False
    )
    ihi_d = nc.declare_dram_parameter(
        "ihi", [GP, 128, T1 * 128 // 16], dt.int16, isOutput=False
    )
    oh_d = nc.declare_dram_parameter("oh", [GP, 128, TT * 128], dt.bfloat16, isOutput=False)
    # Gate matrices stacked for 32-aligned K=32 matmuls: every 32-row window
    # of "a" is [U; 0] (even periods), of "b" is [0; U] (odd periods).
    uza_d = nc.declare_dram_parameter("uza", [128, 128], dt.bfloat16, isOutput=False)
    uzb_d = nc.declare_dram_parameter("uzb", [128, 128], dt.bfloat16, isOutput=False)
    uha_d = nc.declare_dram_parameter("uha", [128, 128], dt.bfloat16, isOutput=False)
    uhb_d = nc.declare_dram_parameter("uhb", [128, 128], dt.bfloat16, isOutput=False)
    cz_d = nc.declare_dram_parameter("cz", [128, 1], dt.float32, isOutput=False)
    ch_d = nc.declare_dram_parameter("ch", [128, 1], dt.float32, isOutput=False)
    wo_d = nc.declare_dram_parameter("wo", [128, 16], dt.bfloat16, isOutput=False)
    bo_d = nc.declare_dram_parameter("bo", [16, 1], dt.float32, isOutput=False)
    pr_d = nc.declare_dram_parameter("pr", [128, 12], dt.float32, isOutput=False)
    id_d = nc.declare_dram_parameter("ident", [128, 128], dt.bfloat16, isOutput=False)
    # own-core X rows (self-loop fast path) + per-node 1/deg
    xo_d = nc.declare_dram_parameter("xown", [GP * 128, 256], dt.bfloat16, isOutput=False)
    d2_d = nc.declare_dram_parameter("d2", [128, GP], dt.float32, isOutput=False)
    out_d = nc.declare_dram_parameter("out", [16, GP * 128], dt.float32, isOutput=True)
    if debug:
        ydbg_d = nc.declare_dram_parameter(
            "ydbg", [GP, 128, 192], dt.bfloat16, isOutput=True
        )
        ytdbg_d = nc.declare_dram_parameter(
            "ytdbg", [NCALLS, 2, 96, GPC * 128], dt.bfloat16, isOutput=True
        )
        hdbg_d = nc.declare_dram_parameter(
            "hdbg", [NCALLS, 128, GPC * 128], dt.float32, isOutput=True
        )

    with tile.TileContext(nc) as tc:
        with (
            tc.tile_pool(name="const", bufs=1) as cpool,
            tc.tile_pool(name="gath", bufs=6) as gpool,
            tc.tile_pool(name="ohp", bufs=4) as opool,
            tc.tile_pool(name="work", bufs=4) as wpool,
            tc.tile_pool(name="acc", bufs=3) as apool,
            tc.tile_pool(name="psy", bufs=2, space="PSUM") as psy,
            tc.tile_pool(name="pst", bufs=2, space="PSUM") as pst,
            tc.tile_pool(name="psd", bufs=2, space="PSUM") as psd,
        ):
            uza_sb = cpool.tile([128, 128], dt.bfloat16)
            nc.sync.dma_start(uza_sb[:], uza_d[:])
            uzb_sb = cpool.tile([128, 128], dt.bfloat16)
            nc.sync.dma_start(uzb_sb[:], uzb_d[:])
            uha_sb = cpool.tile([128, 128], dt.bfloat16)
            nc.sync.dma_start(uha_sb[:], uha_d[:])
            uhb_sb = cpool.tile([128, 128], dt.bfloat16)
            nc.sync.dma_start(uhb_sb[:], uhb_d[:])
            cz_sb = cpool.tile([128, 1], dt.float32)
            nc.sync.dma_start(cz_sb[:], cz_d[:])
            ch_sb = cpool.tile([128, 1], dt.float32)
            nc.sync.dma_start(ch_sb[:], ch_d[:])
            wo_sb = cpool.tile([128, 16], dt.bfloat16)
            nc.sync.dma_start(wo_sb[:], wo_d[:])
            bo_sb = cpool.tile([16, 1], dt.float32)
            nc.sync.dma_start(bo_sb[:], bo_d[:])
            pr_sb = cpool.tile([128, 12], dt.float32)
            nc.sync.dma_start(pr_sb[:], pr_d[:])
            id_sb = cpool.tile([128, 128], dt.bfloat16)
            nc.sync.dma_start(id_sb[:], id_d[:])
            d2_sb = cpool.tile([128, GP], dt.float32)
            nc.sync.dma_start(d2_sb[:], d2_d[:])
            outT = cpool.tile([16, GP * 128], dt.float32)

            qrot = 0
            for c in range(NCALLS):
                # yT tiles for this 4-group batch: [96, 512] bf16, p-major
                # (row 16*(p%6)+f; yT0 holds periods 0-5, yT1 periods 6-11).
                yT0 = wpool.tile([96, GPC * 128], dt.bfloat16, tag="yT0")
                yT1 = wpool.tile([96, GPC * 128], dt.bfloat16, tag="yT1")

                for gi in range(GPC):
                    gslot = c * GPC + gi
                    t0g = t0s[gslot]
                    t1g = t1s[gslot]
                    ttg = t0g + t1g
                    g0 = t0g * 128
                    g1 = t1g * 128
                    glo = gpool.tile([128, t0g, 256], dt.bfloat16, tag="glo")
                    ghi = gpool.tile([128, t1g, 256], dt.bfloat16, tag="ghi")
                    ilo = gpool.tile([128, g0 // 16], dt.int16, tag="ilo")
                    ihi = gpool.tile([128, g1 // 16], dt.int16, tag="ihi")
                    nc.scalar.dma_start(ilo[:], ilo_d[gslot, :, : g0 // 16])
                    nc.scalar.dma_start(ihi[:], ihi_d[gslot, :, : g1 // 16])
                    nc.gpsimd.dma_gather(
                        glo[:], xlo[:, :], ilo[:], g0, g0, 256,
                        single_packet=False, queue_num=qrot % 4,
                    )
                    nc.gpsimd.dma_gather(
                        ghi[:], xhi[:, :], ihi[:], g1, g1, 256,
                        single_packet=False, queue_num=(qrot + 1) % 4,
                    )
                    qrot += 2
                    oh_sb = opool.tile([128, ttg * 128], dt.bfloat16, tag="oh")
                    nc.sync.dma_start(oh_sb[:], oh_d[gslot, :, : ttg * 128])
                    py = psy.tile([128, 192], dt.float32, tag="py")
                    for t in range(ttg):
                        if t < t0g:
                            rhs = glo[:, t, 0:192]
                        else:
                            rhs = ghi[:, t - t0g, 0:192]
                        nc.tensor.matmul(
                            py[:],
                            oh_sb[:, t * 128 : (t + 1) * 128],
                            rhs,
                            start=(t == 0),
                            stop=(t == ttg - 1),
                        )
                    # ysb = py + (1/deg) * X_own  — folds the self-loop term
                    # into the PSUM->SBUF copy (one DVE op, no ACT copy)
                    xot = wpool.tile([128, 192], dt.bfloat16, tag="xot")
                    nc.scalar.dma_start(
                        xot[:], xo_d[gslot * 128 : (gslot + 1) * 128, 0:192]
                    )
                    ysb = wpool.tile([128, 192], dt.bfloat16, tag="ysb")
                    nc.vector.scalar_tensor_tensor(
                        ysb[:], xot[:], d2_sb[:, gslot : gslot + 1], py[:],
                        ALU.mult, ALU.add,
                    )
                    if debug:
                        nc.sync.dma_start(ydbg_d[gslot], ysb[:])
                    ptA = pst.tile([128, 128], dt.bfloat16, tag="pt")
                    nc.tensor.transpose(ptA[0:96, :], ysb[:, 0:96], id_sb[:])
                    nc.scalar.activation(
                        yT0[:, gi * 128 : (gi + 1) * 128], ptA[0:96, :], AF.Copy
                    )
                    ptB = pst.tile([128, 128], dt.bfloat16, tag="pt")
                    nc.tensor.transpose(ptB[0:96, :], ysb[:, 96:192], id_sb[:])
                    nc.scalar.activation(
                        yT1[:, gi * 128 : (gi + 1) * 128], ptB[0:96, :], AF.Copy
                    )

                # Dense GRU over the 512-node batch.
                nodes = GPC * 128
                if debug:
                    nc.sync.dma_start(ytdbg_d[c, 0], yT0[:])
                    nc.sync.dma_start(ytdbg_d[c, 1], yT1[:])
                hacc = apool.tile([128, nodes], dt.float32, tag="hacc")
                for p in range(12):
                    yTt = yT0 if p < 6 else yT1
                    b = 32 * ((p % 6) // 2)
                    uz_t = uza_sb if p % 2 == 0 else uzb_sb
                    uh_t = uha_sb if p % 2 == 0 else uhb_sb
                    pd = psd.tile([128, 2 * nodes], dt.float32, tag="pd")
                    nc.tensor.matmul(
                        pd[:, 0:nodes], uz_t[b : b + 32, :], yTt[b : b + 32, :],
                        start=True, stop=True,
                    )
                    nc.tensor.matmul(
                        pd[:, nodes : 2 * nodes], uh_t[b : b + 32, :], yTt[b : b + 32, :],
                        start=True, stop=True,
                    )
                    zc = wpool.tile([128, nodes], dt.float32, tag="zc")
                    ht = wpool.tile([128, nodes], dt.float32, tag="ht")
                    nc.scalar.activation(
                        zc[:], pd[:, 0:nodes], AF.Sigmoid, bias=cz_sb[:, 0:1]
                    )
                    nc.scalar.activation(
                        ht[:], pd[:, nodes : 2 * nodes], AF.Tanh, bias=ch_sb[:, 0:1]
                    )
                    if p == 0:
                        nc.vector.scalar_tensor_tensor(
                            hacc[:], zc[:], pr_sb[:, p : p + 1], ht[:],
                            ALU.mult, ALU.mult,
                        )
                    else:
                        t2 = wpool.tile([128, nodes], dt.float32, tag="t2")
                        nc.vector.scalar_tensor_tensor(
                            t2[:], zc[:], pr_sb[:, p : p + 1], ht[:],
                            ALU.mult, ALU.mult,
                        )
                        nc.vector.tensor_tensor(hacc[:], hacc[:], t2[:], ALU.add)

                if debug:
                    nc.sync.dma_start(hdbg_d[c], hacc[:])
                # Output: outT[:12, n] = W_out @ relu(Hacc) + b_out
                for gi in range(GPC):
                    gslot = c * GPC + gi
                    hrelu = wpool.tile([128, 128], dt.bfloat16, tag="hrelu")
                    nc.scalar.activation(
                        hrelu[:], hacc[:, gi * 128 : (gi + 1) * 128], AF.Relu
                    )
                    po = pst.tile([16, 128], dt.float32, tag="pt")
                    nc.tensor.matmul(po[:], wo_sb[:], hrelu[:], start=True, stop=True)
                    nc.vector.tensor_scalar_add(
                        outT[:, gslot * 128 : (gslot + 1) * 128], po[:], bo_sb[:, 0:1]
                    )

            nc.sync.dma_start(out_d[:], outT[:])

    if not nc.is_finalized():
        nc.finalize()
    return nc


def kernel(
    x, edge_index, edge_weight, attention,
    W_z, b_z, W_r, b_r, W_h, b_h,
    lin_Wz, lin_bz, lin_Wr, lin_br, lin_Wh, lin_bh,
    W_out, b_out,
):
    global LAST
    x = np.asarray(x, np.float32)
    ei = np.asarray(edge_index, np.int64)
    ew = np.asarray(edge_weight, np.float32)
    W_z = np.asarray(W_z, np.float32)
    b_z = np.asarray(b_z, np.float32)
    W_h = np.asarray(W_h, np.float32)
    b_h = np.asarray(b_h, np.float32)
    lin_Wz = np.asarray(lin_Wz, np.float32)
    lin_bz = np.asarray(lin_bz, np.float32)
    lin_Wh = np.asarray(lin_Wh, np.float32)
    lin_bh = np.asarray(lin_bh, np.float32)
    W_out = np.asarray(W_out, np.float32)
    b_out = np.asarray(b_out, np.float32)

    # ---- fold the GRU algebra into two [16, 128] matrices + biases ----
    probs = _softmax(attention)
    Mz = lin_Wz[:, :O].T
    Uz = -(W_z @ Mz)
    cz = -(b_z @ Mz + lin_bz)
    Mh = lin_Wh[:, :O].T
    Uh = W_h @ Mh
    ch = b_h @ Mh + lin_bh

    # ---- X in period-major layout, bf16, padded to 256 cols ----
    Xp = np.zeros((N, 256), np.float32)
    Xp[:, : F * T] = x.transpose(0, 2, 1).reshape(N, F * T)  # col p*16+f
    Xbf = Xp.astype(BF16)

    # ---- GCN normalization (with self loops) ----
    # deg includes the self loops; the self-loop contribution (X[i]/deg[i])
    # bypasses the gather (contiguous rows, handled on-chip), so the gathered
    # edge stream carries only the real E edges.
    src_f = ei[0]
    dst_f = ei[1]
    deg = (
        np.bincount(dst_f, weights=ew, minlength=N) + 1.0
    ).astype(np.float32)
    dis = (1.0 / np.sqrt(deg)).astype(np.float32)
    norm = dis[src_f] * ew * dis[dst_f]

    # ---- bucket edges by (core, dst group, src half) ----
    core = dst_f // NPC
    rem = dst_f - core * NPC
    g = rem >> 7
    d128 = rem & 127
    h = (src_f >= HALF).astype(np.int64)
    bucket = (core * G + g) * 2 + h
    order = np.argsort(bucket, kind="stable")
    cnt = np.bincount(bucket, minlength=NCORES * G * 2)
    T0 = int(-(-cnt.reshape(-1, 2)[:, 0].max() // 128))
    T1 = int(-(-cnt.reshape(-1, 2)[:, 1].max() // 128))
    TT = T0 + T1
    starts = np.zeros(NCORES * G * 2, np.int64)
    np.cumsum(cnt[:-1], out=starts[1:])
    sb = bucket[order]
    within = np.arange(order.shape[0], dtype=np.int64) - starts[sb]

    # per-group-slot tile counts: max over the 8 cores (static in the SPMD
    # graph, but far tighter than the global max)
    cnt3 = cnt.reshape(NCORES, G, 2)
    t0s = np.maximum(1, -(-cnt3[:, :, 0].max(axis=0) // 128)).astype(np.int64)
    t1s = np.maximum(1, -(-cnt3[:, :, 1].max(axis=0) // 128)).astype(np.int64)
    t0s = np.concatenate([t0s, np.ones(GP - G, np.int64)])
    t1s = np.concatenate([t1s, np.ones(GP - G, np.int64)])

    sc = core[order]
    sg = g[order]
    sh = h[order]
    sd = d128[order]
    ssrc = src_f[order]
    snorm = norm[order]
    tile_of = np.where(sh == 0, within >> 7, t0s[sg] + (within >> 7))
    q = within & 127

    idx_lo = np.zeros((NCORES, GP, T0 * 128), np.int16)
    idx_hi = np.zeros((NCORES, GP, T1 * 128), np.int16)
    m0 = sh == 0
    m1 = ~m0
    idx_lo[sc[m0], sg[m0], within[m0]] = ssrc[m0].astype(np.int16)
    idx_hi[sc[m1], sg[m1], within[m1]] = (ssrc[m1] - HALF).astype(np.int16)
    idx_lo = _wrap16(idx_lo)
    idx_hi = _wrap16(idx_hi)

    oh = np.zeros((NCORES, GP, 128, TT * 128), BF16)
    oh[sc, sg, q, tile_of * 128 + sd] = snorm.astype(BF16)

    # ---- build + run the SPMD graph ----
    nc = _build_graph(
        T0, T1, [int(v) for v in t0s], [int(v) for v in t1s],
        debug=bool(os.environ.get("A3_DEBUG")),
    )

    wo = np.zeros((128, 16), np.float32)
    wo[:, :T] = W_out.T
    bo = np.zeros((16, 1), np.float32)
    bo[:T, 0] = b_out
    uza = np.zeros((128, 128), np.float32)
    uzb = np.zeros((128, 128), np.float32)
    uha = np.zeros((128, 128), np.float32)
    uhb = np.zeros((128, 128), np.float32)
    for j in range(4):
        uza[32 * j : 32 * j + 16] = Uz
        uzb[32 * j + 16 : 32 * j + 32] = Uz
        uha[32 * j : 32 * j + 16] = Uh
        uhb[32 * j + 16 : 32 * j + 32] = Uh
    uza = uza.astype(BF16)
    uzb = uzb.astype(BF16)
    uha = uha.astype(BF16)
    uhb = uhb.astype(BF16)
    czc = np.ascontiguousarray(cz.reshape(128, 1))
    chc = np.ascontiguousarray(ch.reshape(128, 1))
    wobf = wo.astype(BF16)
    prt = np.ascontiguousarray(np.tile(probs, (128, 1)).astype(np.float32))
    ident = np.eye(128, dtype=BF16)
    xlo_a = np.ascontiguousarray(Xbf[:HALF])
    xhi_a = np.ascontiguousarray(Xbf[HALF:])
    xown = np.zeros((NCORES, GP * 128, 256), BF16)
    xown[:, :NPC] = Xbf.reshape(NCORES, NPC, 256)
    d2 = np.zeros((NCORES, GP * 128), np.float32)
    d2[:, :NPC] = (1.0 / deg).reshape(NCORES, NPC)
    d2 = np.ascontiguousarray(
        d2.reshape(NCORES, GP, 128).transpose(0, 2, 1)
    )  # [NCORES, 128, GP]

    in_maps = []
    for k in range(NCORES):
        in_maps.append(
            {
                "xlo": xlo_a,
                "xhi": xhi_a,
                "ilo": idx_lo[k],
                "ihi": idx_hi[k],
                "oh": oh[k],
                "uza": uza,
                "uzb": uzb,
                "uha": uha,
                "uhb": uhb,
                "cz": czc,
                "ch": chc,
                "wo": wobf,
                "bo": bo,
                "pr": prt,
                "ident": ident,
                "xown": xown[k],
                "d2": d2[k],
            }
        )

    LAST = _run(nc, in_maps, trace=bool(os.environ.get("KBENCH_TRACE")))

    full = np.zeros((N, T), np.float32)
    for k in range(NCORES):
        full[k * NPC : (k + 1) * NPC, :] = LAST["results"][k]["out"][:T, :NPC].T
    return full


def _ntff_hook():
    """Contextmanager (dir, device_ids) that captures NTFF profiles via the
    axon PJRT .so, replicating the missing antenv.axon_hooks plumbing."""
    import contextlib
    import ctypes

    so_path = "/opt/axon/libaxon_pjrt.so"
    lib = ctypes.CDLL(so_path)
    if not hasattr(lib, "axon_start_nrt_profile"):
        return None
    lib.axon_start_nrt_profile.argtypes = [
        ctypes.POINTER(ctypes.c_int64),
        ctypes.c_size_t,
    ]
    lib.axon_start_nrt_profile.restype = ctypes.c_int64
    lib.axon_stop_nrt_profile.argtypes = [ctypes.c_char_p]
    lib.axon_stop_nrt_profile.restype = ctypes.c_int64

    @contextlib.contextmanager
    def _hook(output_dir, device_ids):
        import jax

        jax.devices()
        if device_ids:
            ids = (ctypes.c_int64 * len(device_ids))(*device_ids)
            rc = lib.axon_start_nrt_profile(ids, len(device_ids))
        else:
            rc = lib.axon_start_nrt_profile(None, 0)
        if rc != 0:
            raise RuntimeError(f"axon_start_nrt_profile rc={rc}")
        try:
            yield
        finally:
            n = lib.axon_stop_nrt_profile(str(output_dir).encode())
            print(f"ntff profile: {n} file(s) -> {output_dir}")

    return _hook


def _run(nc, in_maps, trace=False):
    import tempfile

    from concourse import bass2jax

    out = {"results": None, "exec_time_ns": None, "trace_path": None}
    if not trace:
        out["results"] = bass2jax.run_bass_via_pjrt(nc, in_maps, n_cores=NCORES)
        return out

    hook = _ntff_hook()
    neff_dir = tempfile.mkdtemp(prefix="a3tgcn_prof_")
    # Single profiled run: re-executing the same NEFF wedges the exec unit
    # (NRT_EXEC_UNIT_UNRECOVERABLE), so no separate warmup.
    with hook(neff_dir, [0]):
        out["results"] = bass2jax.run_bass_via_pjrt(nc, in_maps, n_cores=NCORES)

    try:
        import gauge.profiler as gp
        from concourse._compat import FishPath
        from gauge import trn_perfetto

        prof = gp.Profile(
            profile_path=FishPath(neff_dir),
            kernel_dev_mode=True,
            profile_on_exit=False,
            bass_kernel=nc.m,
            offline_processing=True,
            fname="*_body*",
        )
        prof.convert_ntffs_to_json((0,))
        json_path = prof.json_path(0).path
        insts, trace_path, exec_ns, scopes = trn_perfetto.main(
            json=json_path,
            out_path=os.path.join(neff_dir, "trace.pftrace"),
            kernel_dev_mode=True,
            bass_kernel=nc.m,
        )
        out["exec_time_ns"] = exec_ns
        out["trace_path"] = trace_path
        out["neff_dir"] = neff_dir
        out["scope_times"] = scopes
    except Exception as exc:  # profiling must never break the numerics
        print(f"profiling failed: {exc!r}")
    return out


# revision 63
# speedup vs baseline: 1.0978x; 1.0978x over previous
"""A3TGCN (GCN + GRU-with-zero-state + attention) on 8 TRN2 NeuronCores.

Math (reference collapses because H0 == 0 every period):
    y   = A_norm @ X            # one SpMM, X = x reshaped [N, F*T] (192 cols)
    Zc_p = sigmoid(y_p @ Uz + cz)          # == 1 - Z_p  (weights negated)
    Ht_p = tanh   (y_p @ Uh + ch)
    Hacc = sum_p probs_p * Zc_p * Ht_p
    out  = relu(Hacc) @ W_out.T + b_out    # [N, 12]
where Uz = -(W_z @ lin_Wz[:, :O].T), etc. (GCN weight commutes past A_norm;
R gate multiplies H0 = 0 so it vanishes.)

Sharding: nodes partitioned by destination across 8 cores (graph parallel),
X replicated, no collectives. Edges are sorted by (core, 128-node dst group,
src half), padded to a fixed per-(group,half) tile count so all 8 cores run
one SPMD instruction stream. Per core the SpMM is: dma_gather of bf16 node
rows (512 B each) + PE matmul against a host-built norm-scaled one-hot
(segment sum in PSUM). Dense GRU runs with O on partitions so the gate
biases ride the ScalarEngine's per-partition bias.
"""

import os
import sys

sys.path.insert(0, "/opt/trn_rl_repo")

import numpy as np
import ml_dtypes

BF16 = ml_dtypes.bfloat16

N, F, T, O, E = 50000, 16, 12, 128, 800000
NCORES = 8
NPC = N // NCORES  # 6250 nodes per core
HALF = N // 2  # 25000 (int16 gather index limit)
G = (NPC + 127) // 128  # 49 real dst groups of 128 nodes
GPC = 4  # groups per gather call
GP = ((G + GPC - 1) // GPC) * GPC  # 52 padded group slots
NCALLS = GP // GPC  # 13

LAST = None  # BassKernelResults of the most recent run (test.py reads this)


def _softmax(a):
    a = np.asarray(a, np.float32)
    e = np.exp(a - a.max())
    return e / e.sum()


def _wrap16(v):
    """Flat int16 index vector -> dma_gather layout [128, n//16].

    Index j lives at [j%16, j//16]; the 16-partition block is replicated to
    all 8 Q7-core partition windows because queue q's rx/tx cores (2q, 2q+1)
    each read their own 16-partition window (channels=(queue+1)*2*16).
    """
    n = v.shape[-1]
    w = v.reshape(*v.shape[:-1], n // 16, 16).swapaxes(-1, -2)
    return np.ascontiguousarray(np.concatenate([w] * 8, axis=-2))


def _build_graph(T0, T1, t0s, t1s, debug=False):
    """t0s/t1s: per-group-slot tile counts (max over the 8 cores), so every
    core runs the same instruction stream but pad work tracks the per-group
    max rather than the global max."""
    import concourse.bacc as bacc
    from concourse import mybir, tile

    dt = mybir.dt
    AF = mybir.ActivationFunctionType
    ALU = mybir.AluOpType
    TT = T0 + T1

    nc = bacc.Bacc(None, target_bir_lowering=False, num_swdge_queues=4)

    xlo = nc.declare_dram_parameter("xlo", [HALF, 256], dt.bfloat16, isOutput=False)
    xhi = nc.declare_dram_parameter("xhi", [HALF, 256], dt.bfloat16, isOutput=False)
    ilo_d = nc.declare_dram_parameter(
        "ilo", [GP, 128, T0 * 128 // 16], dt.int16, isOutput=False
    )
    ihi_d = nc.declare_dram_parameter(
        "ihi", [GP, 128, T1 * 128 // 16], dt.int16, isOutput=False
    )
    oh_d = nc.declare_dram_parameter("oh", [GP, 128, TT * 128], dt.bfloat16, isOutput=False)
    # Gate matrices stacked for 32-aligned K=32 matmuls: every 32-row window
    # of "a" is [U; 0] (even periods), of "b" is [0; U] (odd periods).
    uza_d = nc.declare_dram_parameter("uza", [128, 128], dt.bfloat16, isOutput=False)
    uzb_d = nc.declare_dram_parameter("uzb", [128, 128], dt.bfloat16, isOutput=False)
    uha_d = nc.declare_dram_parameter("uha", [128, 128], dt.bfloat16, isOutput=False)
    uhb_d = nc.declare_dram_parameter("uhb", [128, 128], dt.bfloat16, isOutput=False)
    cz_d = nc.declare_dram_parameter("cz", [128, 1], dt.float32, isOutput=False)
    ch_d = nc.declare_dram_parameter("ch", [128, 1], dt.float32, isOutput=False)
    wo_d = nc.declare_dram_parameter("wo", [128, 16], dt.bfloat16, isOutput=False)
    bo_d = nc.declare_dram_parameter("bo", [16, 1], dt.float32, isOutput=False)
    pr_d = nc.declare_dram_parameter("pr", [128, 12], dt.float32, isOutput=False)
    id_d = nc.declare_dram_parameter("ident", [128, 128], dt.bfloat16, isOutput=False)
    # own-core X rows (self-loop fast path) + per-node 1/deg
    xo_d = nc.declare_dram_parameter("xown", [GP * 128, 256], dt.bfloat16, isOutput=False)
    d2_d = nc.declare_dram_parameter("d2", [128, GP], dt.float32, isOutput=False)
    out_d = nc.declare_dram_parameter("out", [16, GP * 128], dt.float32, isOutput=True)
    if debug:
        ydbg_d = nc.declare_dram_parameter(
            "ydbg", [GP, 128, 192], dt.bfloat16, isOutput=True
        )
        ytdbg_d = nc.declare_dram_parameter(
            "ytdbg", [NCALLS, 2, 96, GPC * 128], dt.bfloat16, isOutput=True
        )
        hdbg_d = nc.declare_dram_parameter(
            "hdbg", [NCALLS, 128, GPC * 128], dt.float32, isOutput=True
        )

    with tile.TileContext(nc) as tc:
        with (
            tc.tile_pool(name="const", bufs=1) as cpool,
            tc.tile_pool(name="gath", bufs=6) as gpool,
            tc.tile_pool(name="ohp", bufs=4) as opool,
            tc.tile_pool(name="work", bufs=4) as wpool,
            tc.tile_pool(name="acc", bufs=3) as apool,
            tc.tile_pool(name="psy", bufs=2, space="PSUM") as psy,
            tc.tile_pool(name="pst", bufs=2, space="PSUM") as pst,
            tc.tile_pool(name="psd", bufs=2, space="PSUM") as psd,
        ):
            uza_sb = cpool.tile([128, 128], dt.bfloat16)
            nc.sync.dma_start(uza_sb[:], uza_d[:])
            uzb_sb = cpool.tile([128, 128], dt.bfloat16)
            nc.sync.dma_start(uzb_sb[:], uzb_d[:])
            uha_sb = cpool.tile([128, 128], dt.bfloat16)
            nc.sync.dma_start(uha_sb[:], uha_d[:])
            uhb_sb = cpool.tile([128, 128], dt.bfloat16)
            nc.sync.dma_start(uhb_sb[:], uhb_d[:])
            cz_sb = cpool.tile([128, 1], dt.float32)
            nc.sync.dma_start(cz_sb[:], cz_d[:])
            ch_sb = cpool.tile([128, 1], dt.float32)
            nc.sync.dma_start(ch_sb[:], ch_d[:])
            wo_sb = cpool.tile([128, 16], dt.bfloat16)
            nc.sync.dma_start(wo_sb[:], wo_d[:])
            bo_sb = cpool.tile([16, 1], dt.float32)
            nc.sync.dma_start(bo_sb[:], bo_d[:])
            pr_sb = cpool.tile([128, 12], dt.float32)
            nc.sync.dma_start(pr_sb[:], pr_d[:])
            id_sb = cpool.tile([128, 128], dt.bfloat16)
            nc.sync.dma_start(id_sb[:], id_d[:])
            d2_sb = cpool.tile([128, GP], dt.float32)
            nc.sync.dma_start(d2_sb[:], d2_d[:])
            outT = cpool.tile([16, GP * 128], dt.float32)

            qrot = 0
            for c in range(NCALLS):
                # yT tiles for this 4-group batch: [96, 512] bf16, p-major
                # (row 16*(p%6)+f; yT0 holds periods 0-5, yT1 periods 6-11).
                yT0 = wpool.tile([96, GPC * 128], dt.bfloat16, tag="yT0")
                yT1 = wpool.tile([96, GPC * 128], dt.bfloat16, tag="yT1")

                for gi in range(GPC):
                    gslot = c * GPC + gi
                    t0g = t0s[gslot]
                    t1g = t1s[gslot]
                    ttg = t0g + t1g
                    g0 = t0g * 128
                    g1 = t1g * 128
                    glo = gpool.tile([128, t0g, 256], dt.bfloat16, tag="glo")
                    ghi = gpool.tile([128, t1g, 256], dt.bfloat16, tag="ghi")
                    ilo = gpool.tile([128, g0 // 16], dt.int16, tag="ilo")
                    ihi = gpool.tile([128, g1 // 16], dt.int16, tag="ihi")
                    nc.sync.dma_start(ilo[:], ilo_d[gslot, :, : g0 // 16])
                    nc.sync.dma_start(ihi[:], ihi_d[gslot, :, : g1 // 16])
                    nc.gpsimd.dma_gather(
                        glo[:], xlo[:, :], ilo[:], g0, g0, 256,
                        single_packet=False, queue_num=qrot % 4,
                    )
                    nc.gpsimd.dma_gather(
                        ghi[:], xhi[:, :], ihi[:], g1, g1, 256,
                        single_packet=False, queue_num=(qrot + 1) % 4,
                    )
                    qrot += 2
                    oh_sb = opool.tile([128, ttg * 128], dt.bfloat16, tag="oh")
                    nc.sync.dma_start(oh_sb[:], oh_d[gslot, :, : ttg * 128])
                    py = psy.tile([128, 192], dt.float32, tag="py")
                    for t in range(ttg):
                        if t < t0g:
                            rhs = glo[:, t, 0:192]
                        else:
                            rhs = ghi[:, t - t0g, 0:192]
                        nc.tensor.matmul(
                            py[:],
                            oh_sb[:, t * 128 : (t + 1) * 128],
                            rhs,
                            start=(t == 0),
                            stop=(t == ttg - 1),
                        )
                    # ysb = py + (1/deg) * X_own  — folds the self-loop term
                    # into the PSUM->SBUF copy (one DVE op, no ACT copy)
                    xot = wpool.tile([128, 192], dt.bfloat16, tag="xot")
                    nc.sync.dma_start(
                        xot[:], xo_d[gslot * 128 : (gslot + 1) * 128, 0:192]
                    )
                    ysb = wpool.tile([128, 192], dt.bfloat16, tag="ysb")
                    nc.vector.scalar_tensor_tensor(
                        ysb[:], xot[:], d2_sb[:, gslot : gslot + 1], py[:],
                        ALU.mult, ALU.add,
                    )
                    if debug:
                        nc.sync.dma_start(ydbg_d[gslot], ysb[:])
                    ptA = pst.tile([128, 128], dt.bfloat16, tag="pt")
                    nc.tensor.transpose(ptA[0:96, :], ysb[:, 0:96], id_sb[:])
                    nc.scalar.activation(
                        yT0[:, gi * 128 : (gi + 1) * 128], ptA[0:96, :], AF.Copy
                    )
                    ptB = pst.tile([128, 128], dt.bfloat16, tag="pt")
                    nc.tensor.transpose(ptB[0:96, :], ysb[:, 96:192], id_sb[:])
                    nc.scalar.activation(
                        yT1[:, gi * 128 : (gi + 1) * 128], ptB[0:96, :], AF.Copy
                    )

                # Dense GRU over the 512-node batch.
                nodes = GPC * 128
                if debug:
                    nc.sync.dma_start(ytdbg_d[c, 0], yT0[:])
                    nc.sync.dma_start(ytdbg_d[c, 1], yT1[:])
                hacc = apool.tile([128, nodes], dt.float32, tag="hacc")
                for p in range(12):
                    yTt = yT0 if p < 6 else yT1
                    b = 32 * ((p % 6) // 2)
                    uz_t = uza_sb if p % 2 == 0 else uzb_sb
                    uh_t = uha_sb if p % 2 == 0 else uhb_sb
                    pd = psd.tile([128, 2 * nodes], dt.float32, tag="pd")
                    nc.tensor.matmul(
                        pd[:, 0:nodes], uz_t[b : b + 32, :], yTt[b : b + 32, :],
                        start=True, stop=True,
                    )
                    nc.tensor.matmul(
                        pd[:, nodes : 2 * nodes], uh_t[b : b + 32, :], yTt[b : b + 32, :],
                        start=True, stop=True,
                    )
                    zc = wpool.tile([128, nodes], dt.float32, tag="zc")
                    ht = wpool.tile([128, nodes], dt.float32, tag="ht")
                    nc.scalar.activation(
                        zc[:], pd[:, 0:nodes], AF.Sigmoid, bias=cz_sb[:, 0:1]
                    )
                    nc.scalar.activation(
                        ht[:], pd[:, nodes : 2 * nodes], AF.Tanh, bias=ch_sb[:, 0:1]
                    )
                    if p == 0:
                        nc.vector.scalar_tensor_tensor(
                            hacc[:], zc[:], pr_sb[:, p : p + 1], ht[:],
                            ALU.mult, ALU.mult,
                        )
                    else:
                        t2 = wpool.tile([128, nodes], dt.float32, tag="t2")
                        nc.vector.scalar_tensor_tensor(
                            t2[:], zc[:], pr_sb[:, p : p + 1], ht[:],
                            ALU.mult, ALU.mult,
                        )
                        nc.vector.tensor_tensor(hacc[:], hacc[:], t2[:], ALU.add)

                if debug:
                    nc.sync.dma_start(hdbg_d[c], hacc[:])
                # Output: outT[:12, n] = W_out @ relu(Hacc) + b_out
                for gi in range(GPC):
                    gslot = c * GPC + gi
                    hrelu = wpool.tile([128, 128], dt.bfloat16, tag="hrelu")
                    nc.scalar.activation(
                        hrelu[:], hacc[:, gi * 128 : (gi + 1) * 128], AF.Relu
                    )
                    po = pst.tile([16, 128], dt.float32, tag="pt")
                    nc.tensor.matmul(po[:], wo_sb[:], hrelu[:], start=True, stop=True)
                    nc.vector.tensor_scalar_add(
                        outT[:, gslot * 128 : (gslot + 1) * 128], po[:], bo_sb[:, 0:1]
                    )

            nc.sync.dma_start(out_d[:], outT[:])

    if not nc.is_finalized():
        nc.finalize()
    return nc


def kernel(
    x, edge_index, edge_weight, attention,
    W_z, b_z, W_r, b_r, W_h, b_h,
    lin_Wz, lin_bz, lin_Wr, lin_br, lin_Wh, lin_bh,
    W_out, b_out,
):
    global LAST
    x = np.asarray(x, np.float32)
    ei = np.asarray(edge_index, np.int64)
    ew = np.asarray(edge_weight, np.float32)
    W_z = np.asarray(W_z, np.float32)
    b_z = np.asarray(b_z, np.float32)
    W_h = np.asarray(W_h, np.float32)
    b_h = np.asarray(b_h, np.float32)
    lin_Wz = np.asarray(lin_Wz, np.float32)
    lin_bz = np.asarray(lin_bz, np.float32)
    lin_Wh = np.asarray(lin_Wh, np.float32)
    lin_bh = np.asarray(lin_bh, np.float32)
    W_out = np.asarray(W_out, np.float32)
    b_out = np.asarray(b_out, np.float32)

    # ---- fold the GRU algebra into two [16, 128] matrices + biases ----
    probs = _softmax(attention)
    Mz = lin_Wz[:, :O].T
    Uz = -(W_z @ Mz)
    cz = -(b_z @ Mz + lin_bz)
    Mh = lin_Wh[:, :O].T
    Uh = W_h @ Mh
    ch = b_h @ Mh + lin_bh

    # ---- X in period-major layout, bf16, padded to 256 cols ----
    Xp = np.zeros((N, 256), np.float32)
    Xp[:, : F * T] = x.transpose(0, 2, 1).reshape(N, F * T)  # col p*16+f
    Xbf = Xp.astype(BF16)

    # ---- GCN normalization (with self loops) ----
    # deg includes the self loops; the self-loop contribution (X[i]/deg[i])
    # bypasses the gather (contiguous rows, handled on-chip), so the gathered
    # edge stream carries only the real E edges.
    src_f = ei[0]
    dst_f = ei[1]
    deg = (
        np.bincount(dst_f, weights=ew, minlength=N) + 1.0
    ).astype(np.float32)
    dis = (1.0 / np.sqrt(deg)).astype(np.float32)
    norm = dis[src_f] * ew * dis[dst_f]

    # ---- bucket edges by (core, dst group, src half) ----
    core = dst_f // NPC
    rem = dst_f - core * NPC
    g = rem >> 7
    d128 = rem & 127
    h = (src_f >= HALF).astype(np.int64)
    bucket = (core * G + g) * 2 + h
    order = np.argsort(bucket, kind="stable")
    cnt = np.bincount(bucket, minlength=NCORES * G * 2)
    T0 = int(-(-cnt.reshape(-1, 2)[:, 0].max() // 128))
    T1 = int(-(-cnt.reshape(-1, 2)[:, 1].max() // 128))
    TT = T0 + T1
    starts = np.zeros(NCORES * G * 2, np.int64)
    np.cumsum(cnt[:-1], out=starts[1:])
    sb = bucket[order]
    within = np.arange(order.shape[0], dtype=np.int64) - starts[sb]

    # per-group-slot tile counts: max over the 8 cores (static in the SPMD
    # graph, but far tighter than the global max)
    cnt3 = cnt.reshape(NCORES, G, 2)
    t0s = np.maximum(1, -(-cnt3[:, :, 0].max(axis=0) // 128)).astype(np.int64)
    t1s = np.maximum(1, -(-cnt3[:, :, 1].max(axis=0) // 128)).astype(np.int64)
    t0s = np.concatenate([t0s, np.ones(GP - G, np.int64)])
    t1s = np.concatenate([t1s, np.ones(GP - G, np.int64)])

    sc = core[order]
    sg = g[order]
    sh = h[order]
    sd = d128[order]
    ssrc = src_f[order]
    snorm = norm[order]
    tile_of = np.where(sh == 0, within >> 7, t0s[sg] + (within >> 7))
    q = within & 127

    idx_lo = np.zeros((NCORES, GP, T0 * 128), np.int16)
    idx_hi = np.zeros((NCORES, GP, T1 * 128), np.int16)
    m0 = sh == 0
    m1 = ~m0
    idx_lo[sc[m0], sg[m0], within[m0]] = ssrc[m0].astype(np.int16)
    idx_hi[sc[m1], sg[m1], within[m1]] = (ssrc[m1] - HALF).astype(np.int16)
    idx_lo = _wrap16(idx_lo)
    idx_hi = _wrap16(idx_hi)

    oh = np.zeros((NCORES, GP, 128, TT * 128), BF16)
    oh[sc, sg, q, tile_of * 128 + sd] = snorm.astype(BF16)

    # ---- build + run the SPMD graph ----
    nc = _build_graph(
        T0, T1, [int(v) for v in t0s], [int(v) for v in t1s],
        debug=bool(os.environ.get("A3_DEBUG")),
    )

    wo = np.zeros((128, 16), np.float32)
    wo[:, :T] = W_out.T
    bo = np.zeros((16, 1), np.float32)
    bo[:T, 0] = b_out
    uza = np.zeros((128, 128), np.float32)
    uzb = np.zeros((128, 128), np.float32)
    uha = np.zeros((128, 128), np.float32)
    uhb = np.zeros((128, 128), np.float32)
    for j in range(4):
        uza[32 * j : 32 * j + 16] = Uz
        uzb[32 * j + 16 : 32 * j + 32] = Uz
        uha[32 * j : 32 * j + 16] = Uh
        uhb[32 * j + 16 : 32 * j + 32] = Uh
    uza = uza.astype(BF16)
    uzb = uzb.astype(BF16)
    uha = uha.astype(BF16)
    uhb = uhb.astype(BF16)
    czc = np.ascontiguousarray(cz.reshape(128, 1))
    chc = np.ascontiguousarray(ch.reshape(128, 1))
    wobf = wo.astype(BF16)
    prt = np.ascontiguousarray(np.tile(probs, (128, 1)).astype(np.float32))
    ident = np.eye(128, dtype=BF16)
    xlo_a = np.ascontiguousarray(Xbf[:HALF])
    xhi_a = np.ascontiguousarray(Xbf[HALF:])
    xown = np.zeros((NCORES, GP * 128, 256), BF16)
    xown[:, :NPC] = Xbf.reshape(NCORES, NPC, 256)
    d2 = np.zeros((NCORES, GP * 128), np.float32)
    d2[:, :NPC] = (1.0 / deg).reshape(NCORES, NPC)
    d2 = np.ascontiguousarray(
        d2.reshape(NCORES, GP, 128).transpose(0, 2, 1)
    )  # [NCORES, 128, GP]

    in_maps = []
    for k in range(NCORES):
        in_maps.append(
            {
                "xlo": xlo_a,
                "xhi": xhi_a,
                "ilo": idx_lo[k],
                "ihi": idx_hi[k],
                "oh": oh[k],
                "uza": uza,
                "uzb": uzb,
                "uha": uha,
                "uhb": uhb,
                "cz": czc,
                "ch": chc,
                "wo": wobf,
                "bo": bo,
                "pr": prt,
                "ident": ident,
                "xown": xown[k],
                "d2": d2[k],
            }
        )

    LAST = _run(nc, in_maps, trace=bool(os.environ.get("KBENCH_TRACE")))

    full = np.zeros((N, T), np.float32)
    for k in range(NCORES):
        full[k * NPC : (k + 1) * NPC, :] = LAST["results"][k]["out"][:T, :NPC].T
    return full


def _ntff_hook():
    """Contextmanager (dir, device_ids) that captures NTFF profiles via the
    axon PJRT .so, replicating the missing antenv.axon_hooks plumbing."""
    import contextlib
    import ctypes

    so_path = "/opt/axon/libaxon_pjrt.so"
    lib = ctypes.CDLL(so_path)
    if not hasattr(lib, "axon_start_nrt_profile"):
        return None
    lib.axon_start_nrt_profile.argtypes = [
        ctypes.POINTER(ctypes.c_int64),
        ctypes.c_size_t,
    ]
    lib.axon_start_nrt_profile.restype = ctypes.c_int64
    lib.axon_stop_nrt_profile.argtypes = [ctypes.c_char_p]
    lib.axon_stop_nrt_profile.restype = ctypes.c_int64

    @contextlib.contextmanager
    def _hook(output_dir, device_ids):
        import jax

        jax.devices()
        if device_ids:
            ids = (ctypes.c_int64 * len(device_ids))(*device_ids)
            rc = lib.axon_start_nrt_profile(ids, len(device_ids))
        else:
            rc = lib.axon_start_nrt_profile(None, 0)
        if rc != 0:
            raise RuntimeError(f"axon_start_nrt_profile rc={rc}")
        try:
            yield
        finally:
            n = lib.axon_stop_nrt_profile(str(output_dir).encode())
            print(f"ntff profile: {n} file(s) -> {output_dir}")

    return _hook


def _run(nc, in_maps, trace=False):
    import tempfile

    from concourse import bass2jax

    out = {"results": None, "exec_time_ns": None, "trace_path": None}
    if not trace:
        out["results"] = bass2jax.run_bass_via_pjrt(nc, in_maps, n_cores=NCORES)
        return out

    hook = _ntff_hook()
    neff_dir = tempfile.mkdtemp(prefix="a3tgcn_prof_")
    # Single profiled run: re-executing the same NEFF wedges the exec unit
    # (NRT_EXEC_UNIT_UNRECOVERABLE), so no separate warmup.
    with hook(neff_dir, [0]):
        out["results"] = bass2jax.run_bass_via_pjrt(nc, in_maps, n_cores=NCORES)

    try:
        import gauge.profiler as gp
        from concourse._compat import FishPath
        from gauge import trn_perfetto

        prof = gp.Profile(
            profile_path=FishPath(neff_dir),
            kernel_dev_mode=True,
            profile_on_exit=False,
            bass_kernel=nc.m,
            offline_processing=True,
            fname="*_body*",
        )
        prof.convert_ntffs_to_json((0,))
        json_path = prof.json_path(0).path
        insts, trace_path, exec_ns, scopes = trn_perfetto.main(
            json=json_path,
            out_path=os.path.join(neff_dir, "trace.pftrace"),
            kernel_dev_mode=True,
            bass_kernel=nc.m,
        )
        out["exec_time_ns"] = exec_ns
        out["trace_path"] = trace_path
        out["neff_dir"] = neff_dir
        out["scope_times"] = scopes
    except Exception as exc:  # profiling must never break the numerics
        print(f"profiling failed: {exc!r}")
    return out


# revision 64
# speedup vs baseline: 1.1970x; 1.0904x over previous
"""A3TGCN (GCN + GRU-with-zero-state + attention) on 8 TRN2 NeuronCores.

Math (reference collapses because H0 == 0 every period):
    y   = A_norm @ X            # one SpMM, X = x reshaped [N, F*T] (192 cols)
    Zc_p = sigmoid(y_p @ Uz + cz)          # == 1 - Z_p  (weights negated)
    Ht_p = tanh   (y_p @ Uh + ch)
    Hacc = sum_p probs_p * Zc_p * Ht_p
    out  = relu(Hacc) @ W_out.T + b_out    # [N, 12]
where Uz = -(W_z @ lin_Wz[:, :O].T), etc. (GCN weight commutes past A_norm;
R gate multiplies H0 = 0 so it vanishes.)

Sharding: nodes partitioned by destination across 8 cores (graph parallel),
X replicated, no collectives. Edges are sorted by (core, 128-node dst group,
src half), padded to a fixed per-(group,half) tile count so all 8 cores run
one SPMD instruction stream. Per core the SpMM is: dma_gather of bf16 node
rows (512 B each) + PE matmul against a host-built norm-scaled one-hot
(segment sum in PSUM). Dense GRU runs with O on partitions so the gate
biases ride the ScalarEngine's per-partition bias.
"""

import os
import sys

sys.path.insert(0, "/opt/trn_rl_repo")

import numpy as np
import ml_dtypes

BF16 = ml_dtypes.bfloat16

N, F, T, O, E = 50000, 16, 12, 128, 800000
NCORES = 8
NPC = N // NCORES  # 6250 nodes per core
HALF = N // 2  # 25000 (int16 gather index limit)
G = (NPC + 127) // 128  # 49 real dst groups of 128 nodes
GPC = 4  # groups per gather call
GP = ((G + GPC - 1) // GPC) * GPC  # 52 padded group slots
NCALLS = GP // GPC  # 13

LAST = None  # BassKernelResults of the most recent run (test.py reads this)


def _softmax(a):
    a = np.asarray(a, np.float32)
    e = np.exp(a - a.max())
    return e / e.sum()


def _wrap16(v):
    """Flat int16 index vector -> dma_gather layout [128, n//16].

    Index j lives at [j%16, j//16]; the 16-partition block is replicated to
    all 8 Q7-core partition windows because queue q's rx/tx cores (2q, 2q+1)
    each read their own 16-partition window (channels=(queue+1)*2*16).
    """
    n = v.shape[-1]
    w = v.reshape(*v.shape[:-1], n // 16, 16).swapaxes(-1, -2)
    return np.ascontiguousarray(np.concatenate([w] * 8, axis=-2))


def _build_graph(T0, T1, t0s, t1s, debug=False):
    """t0s/t1s: per-group-slot tile counts (max over the 8 cores), so every
    core runs the same instruction stream but pad work tracks the per-group
    max rather than the global max."""
    import concourse.bacc as bacc
    from concourse import mybir, tile

    dt = mybir.dt
    AF = mybir.ActivationFunctionType
    ALU = mybir.AluOpType
    TT = T0 + T1

    nc = bacc.Bacc(None, target_bir_lowering=False, num_swdge_queues=4)

    xlo = nc.declare_dram_parameter("xlo", [HALF, 256], dt.bfloat16, isOutput=False)
    xhi = nc.declare_dram_parameter("xhi", [HALF, 256], dt.bfloat16, isOutput=False)
    ilo_d = nc.declare_dram_parameter(
        "ilo", [GP, 128, T0 * 128 // 16], dt.int16, isOutput=False
    )
    ihi_d = nc.declare_dram_parameter(
        "ihi", [GP, 128, T1 * 128 // 16], dt.int16, isOutput=False
    )
    oh_d = nc.declare_dram_parameter("oh", [GP, 128, TT * 128], dt.bfloat16, isOutput=False)
    # Gate matrices stacked for 32-aligned K=32 matmuls: every 32-row window
    # of "a" is [U; 0] (even periods), of "b" is [0; U] (odd periods).
    uza_d = nc.declare_dram_parameter("uza", [128, 128], dt.bfloat16, isOutput=False)
    uzb_d = nc.declare_dram_parameter("uzb", [128, 128], dt.bfloat16, isOutput=False)
    uha_d = nc.declare_dram_parameter("uha", [128, 128], dt.bfloat16, isOutput=False)
    uhb_d = nc.declare_dram_parameter("uhb", [128, 128], dt.bfloat16, isOutput=False)
    cz_d = nc.declare_dram_parameter("cz", [128, 1], dt.float32, isOutput=False)
    ch_d = nc.declare_dram_parameter("ch", [128, 1], dt.float32, isOutput=False)
    wo_d = nc.declare_dram_parameter("wo", [128, 16], dt.bfloat16, isOutput=False)
    bo_d = nc.declare_dram_parameter("bo", [16, 1], dt.float32, isOutput=False)
    pr_d = nc.declare_dram_parameter("pr", [128, 12], dt.float32, isOutput=False)
    id_d = nc.declare_dram_parameter("ident", [128, 128], dt.bfloat16, isOutput=False)
    # own-core X rows (self-loop fast path) + per-node 1/deg
    xo_d = nc.declare_dram_parameter("xown", [GP * 128, 256], dt.bfloat16, isOutput=False)
    d2_d = nc.declare_dram_parameter("d2", [128, GP], dt.float32, isOutput=False)
    out_d = nc.declare_dram_parameter("out", [16, GP * 128], dt.float32, isOutput=True)
    if debug:
        ydbg_d = nc.declare_dram_parameter(
            "ydbg", [GP, 128, 192], dt.bfloat16, isOutput=True
        )
        ytdbg_d = nc.declare_dram_parameter(
            "ytdbg", [NCALLS, 2, 96, GPC * 128], dt.bfloat16, isOutput=True
        )
        hdbg_d = nc.declare_dram_parameter(
            "hdbg", [NCALLS, 128, GPC * 128], dt.float32, isOutput=True
        )

    with tile.TileContext(nc) as tc:
        with (
            tc.tile_pool(name="const", bufs=1) as cpool,
            tc.tile_pool(name="gath", bufs=6) as gpool,
            tc.tile_pool(name="ohp", bufs=3) as opool,
            tc.tile_pool(name="work", bufs=3) as wpool,
            tc.tile_pool(name="acc", bufs=2) as apool,
            tc.tile_pool(name="psy", bufs=2, space="PSUM") as psy,
            tc.tile_pool(name="pst", bufs=2, space="PSUM") as pst,
            tc.tile_pool(name="psd", bufs=2, space="PSUM") as psd,
        ):
            uza_sb = cpool.tile([128, 128], dt.bfloat16)
            nc.sync.dma_start(uza_sb[:], uza_d[:])
            uzb_sb = cpool.tile([128, 128], dt.bfloat16)
            nc.sync.dma_start(uzb_sb[:], uzb_d[:])
            uha_sb = cpool.tile([128, 128], dt.bfloat16)
            nc.sync.dma_start(uha_sb[:], uha_d[:])
            uhb_sb = cpool.tile([128, 128], dt.bfloat16)
            nc.sync.dma_start(uhb_sb[:], uhb_d[:])
            cz_sb = cpool.tile([128, 1], dt.float32)
            nc.sync.dma_start(cz_sb[:], cz_d[:])
            ch_sb = cpool.tile([128, 1], dt.float32)
            nc.sync.dma_start(ch_sb[:], ch_d[:])
            wo_sb = cpool.tile([128, 16], dt.bfloat16)
            nc.sync.dma_start(wo_sb[:], wo_d[:])
            bo_sb = cpool.tile([16, 1], dt.float32)
            nc.sync.dma_start(bo_sb[:], bo_d[:])
            pr_sb = cpool.tile([128, 12], dt.float32)
            nc.sync.dma_start(pr_sb[:], pr_d[:])
            id_sb = cpool.tile([128, 128], dt.bfloat16)
            nc.sync.dma_start(id_sb[:], id_d[:])
            d2_sb = cpool.tile([128, GP], dt.float32)
            nc.sync.dma_start(d2_sb[:], d2_d[:])
            outT = cpool.tile([16, GP * 128], dt.float32)

            qrot = 0
            for c in range(NCALLS):
                # yT tiles for this 4-group batch: [96, 512] bf16, p-major
                # (row 16*(p%6)+f; yT0 holds periods 0-5, yT1 periods 6-11).
                yT0 = wpool.tile([96, GPC * 128], dt.bfloat16, tag="yT0")
                yT1 = wpool.tile([96, GPC * 128], dt.bfloat16, tag="yT1")

                for gi in range(GPC):
                    gslot = c * GPC + gi
                    t0g = t0s[gslot]
                    t1g = t1s[gslot]
                    ttg = t0g + t1g
                    g0 = t0g * 128
                    g1 = t1g * 128
                    glo = gpool.tile([128, t0g, 256], dt.bfloat16, tag="glo")
                    ghi = gpool.tile([128, t1g, 256], dt.bfloat16, tag="ghi")
                    ilo = gpool.tile([128, g0 // 16], dt.int16, tag="ilo")
                    ihi = gpool.tile([128, g1 // 16], dt.int16, tag="ihi")
                    nc.sync.dma_start(ilo[:], ilo_d[gslot, :, : g0 // 16])
                    nc.sync.dma_start(ihi[:], ihi_d[gslot, :, : g1 // 16])
                    nc.gpsimd.dma_gather(
                        glo[:], xlo[:, :], ilo[:], g0, g0, 256,
                        single_packet=False, queue_num=qrot % 4,
                    )
                    nc.gpsimd.dma_gather(
                        ghi[:], xhi[:, :], ihi[:], g1, g1, 256,
                        single_packet=False, queue_num=(qrot + 1) % 4,
                    )
                    qrot += 2
                    oh_sb = opool.tile([128, ttg * 128], dt.bfloat16, tag="oh")
                    nc.sync.dma_start(oh_sb[:], oh_d[gslot, :, : ttg * 128])
                    py = psy.tile([128, 192], dt.float32, tag="py")
                    for t in range(ttg):
                        if t < t0g:
                            rhs = glo[:, t, 0:192]
                        else:
                            rhs = ghi[:, t - t0g, 0:192]
                        nc.tensor.matmul(
                            py[:],
                            oh_sb[:, t * 128 : (t + 1) * 128],
                            rhs,
                            start=(t == 0),
                            stop=(t == ttg - 1),
                        )
                    # ysb = py + (1/deg) * X_own  — folds the self-loop term
                    # into the PSUM->SBUF copy (one DVE op, no ACT copy)
                    xot = wpool.tile([128, 192], dt.bfloat16, tag="xot")
                    nc.sync.dma_start(
                        xot[:], xo_d[gslot * 128 : (gslot + 1) * 128, 0:192]
                    )
                    ysb = wpool.tile([128, 192], dt.bfloat16, tag="ysb")
                    nc.vector.scalar_tensor_tensor(
                        ysb[:], xot[:], d2_sb[:, gslot : gslot + 1], py[:],
                        ALU.mult, ALU.add,
                    )
                    if debug:
                        nc.sync.dma_start(ydbg_d[gslot], ysb[:])
                    ptA = pst.tile([128, 128], dt.bfloat16, tag="pt")
                    nc.tensor.transpose(ptA[0:96, :], ysb[:, 0:96], id_sb[:])
                    nc.scalar.activation(
                        yT0[:, gi * 128 : (gi + 1) * 128], ptA[0:96, :], AF.Copy
                    )
                    ptB = pst.tile([128, 128], dt.bfloat16, tag="pt")
                    nc.tensor.transpose(ptB[0:96, :], ysb[:, 96:192], id_sb[:])
                    nc.scalar.activation(
                        yT1[:, gi * 128 : (gi + 1) * 128], ptB[0:96, :], AF.Copy
                    )

                # Dense GRU over the 512-node batch.
                nodes = GPC * 128
                if debug:
                    nc.sync.dma_start(ytdbg_d[c, 0], yT0[:])
                    nc.sync.dma_start(ytdbg_d[c, 1], yT1[:])
                hacc = apool.tile([128, nodes], dt.float32, tag="hacc")
                for p in range(12):
                    yTt = yT0 if p < 6 else yT1
                    b = 32 * ((p % 6) // 2)
                    uz_t = uza_sb if p % 2 == 0 else uzb_sb
                    uh_t = uha_sb if p % 2 == 0 else uhb_sb
                    pd = psd.tile([128, 2 * nodes], dt.float32, tag="pd")
                    nc.tensor.matmul(
                        pd[:, 0:nodes], uz_t[b : b + 32, :], yTt[b : b + 32, :],
                        start=True, stop=True,
                    )
                    nc.tensor.matmul(
                        pd[:, nodes : 2 * nodes], uh_t[b : b + 32, :], yTt[b : b + 32, :],
                        start=True, stop=True,
                    )
                    zc = wpool.tile([128, nodes], dt.float32, tag="zc")
                    ht = wpool.tile([128, nodes], dt.float32, tag="ht")
                    nc.scalar.activation(
                        zc[:], pd[:, 0:nodes], AF.Sigmoid, bias=cz_sb[:, 0:1]
                    )
                    nc.scalar.activation(
                        ht[:], pd[:, nodes : 2 * nodes], AF.Tanh, bias=ch_sb[:, 0:1]
                    )
                    if p == 0:
                        nc.vector.scalar_tensor_tensor(
                            hacc[:], zc[:], pr_sb[:, p : p + 1], ht[:],
                            ALU.mult, ALU.mult,
                        )
                    else:
                        t2 = wpool.tile([128, nodes], dt.float32, tag="t2")
                        nc.vector.scalar_tensor_tensor(
                            t2[:], zc[:], pr_sb[:, p : p + 1], ht[:],
                            ALU.mult, ALU.mult,
                        )
                        nc.vector.tensor_tensor(hacc[:], hacc[:], t2[:], ALU.add)

                if debug:
                    nc.sync.dma_start(hdbg_d[c], hacc[:])
                # Output: outT[:12, n] = W_out @ relu(Hacc) + b_out
                for gi in range(GPC):
                    gslot = c * GPC + gi
                    hrelu = wpool.tile([128, 128], dt.bfloat16, tag="hrelu")
                    nc.scalar.activation(
                        hrelu[:], hacc[:, gi * 128 : (gi + 1) * 128], AF.Relu
                    )
                    po = pst.tile([16, 128], dt.float32, tag="pt")
                    nc.tensor.matmul(po[:], wo_sb[:], hrelu[:], start=True, stop=True)
                    nc.vector.tensor_scalar_add(
                        outT[:, gslot * 128 : (gslot + 1) * 128], po[:], bo_sb[:, 0:1]
                    )

            nc.sync.dma_start(out_d[:], outT[:])

    if not nc.is_finalized():
        nc.finalize()
    return nc


def kernel(
    x, edge_index, edge_weight, attention,
    W_z, b_z, W_r, b_r, W_h, b_h,
    lin_Wz, lin_bz, lin_Wr, lin_br, lin_Wh, lin_bh,
    W_out, b_out,
):
    global LAST
    x = np.asarray(x, np.float32)
    ei = np.asarray(edge_index, np.int64)
    ew = np.asarray(edge_weight, np.float32)
    W_z = np.asarray(W_z, np.float32)
    b_z = np.asarray(b_z, np.float32)
    W_h = np.asarray(W_h, np.float32)
    b_h = np.asarray(b_h, np.float32)
    lin_Wz = np.asarray(lin_Wz, np.float32)
    lin_bz = np.asarray(lin_bz, np.float32)
    lin_Wh = np.asarray(lin_Wh, np.float32)
    lin_bh = np.asarray(lin_bh, np.float32)
    W_out = np.asarray(W_out, np.float32)
    b_out = np.asarray(b_out, np.float32)

    # ---- fold the GRU algebra into two [16, 128] matrices + biases ----
    probs = _softmax(attention)
    Mz = lin_Wz[:, :O].T
    Uz = -(W_z @ Mz)
    cz = -(b_z @ Mz + lin_bz)
    Mh = lin_Wh[:, :O].T
    Uh = W_h @ Mh
    ch = b_h @ Mh + lin_bh

    # ---- X in period-major layout, bf16, padded to 256 cols ----
    Xp = np.zeros((N, 256), np.float32)
    Xp[:, : F * T] = x.transpose(0, 2, 1).reshape(N, F * T)  # col p*16+f
    Xbf = Xp.astype(BF16)

    # ---- GCN normalization (with self loops) ----
    # deg includes the self loops; the self-loop contribution (X[i]/deg[i])
    # bypasses the gather (contiguous rows, handled on-chip), so the gathered
    # edge stream carries only the real E edges.
    src_f = ei[0]
    dst_f = ei[1]
    deg = (
        np.bincount(dst_f, weights=ew, minlength=N) + 1.0
    ).astype(np.float32)
    dis = (1.0 / np.sqrt(deg)).astype(np.float32)
    norm = dis[src_f] * ew * dis[dst_f]

    # ---- bucket edges by (core, dst group, src half) ----
    core = dst_f // NPC
    rem = dst_f - core * NPC
    g = rem >> 7
    d128 = rem & 127
    h = (src_f >= HALF).astype(np.int64)
    bucket = (core * G + g) * 2 + h
    order = np.argsort(bucket, kind="stable")
    cnt = np.bincount(bucket, minlength=NCORES * G * 2)
    T0 = int(-(-cnt.reshape(-1, 2)[:, 0].max() // 128))
    T1 = int(-(-cnt.reshape(-1, 2)[:, 1].max() // 128))
    TT = T0 + T1
    starts = np.zeros(NCORES * G * 2, np.int64)
    np.cumsum(cnt[:-1], out=starts[1:])
    sb = bucket[order]
    within = np.arange(order.shape[0], dtype=np.int64) - starts[sb]

    # per-group-slot tile counts: max over the 8 cores (static in the SPMD
    # graph, but far tighter than the global max)
    cnt3 = cnt.reshape(NCORES, G, 2)
    t0s = np.maximum(1, -(-cnt3[:, :, 0].max(axis=0) // 128)).astype(np.int64)
    t1s = np.maximum(1, -(-cnt3[:, :, 1].max(axis=0) // 128)).astype(np.int64)
    t0s = np.concatenate([t0s, np.ones(GP - G, np.int64)])
    t1s = np.concatenate([t1s, np.ones(GP - G, np.int64)])

    sc = core[order]
    sg = g[order]
    sh = h[order]
    sd = d128[order]
    ssrc = src_f[order]
    snorm = norm[order]
    tile_of = np.where(sh == 0, within >> 7, t0s[sg] + (within >> 7))
    q = within & 127

    idx_lo = np.zeros((NCORES, GP, T0 * 128), np.int16)
    idx_hi = np.zeros((NCORES, GP, T1 * 128), np.int16)
    m0 = sh == 0
    m1 = ~m0
    idx_lo[sc[m0], sg[m0], within[m0]] = ssrc[m0].astype(np.int16)
    idx_hi[sc[m1], sg[m1], within[m1]] = (ssrc[m1] - HALF).astype(np.int16)
    idx_lo = _wrap16(idx_lo)
    idx_hi = _wrap16(idx_hi)

    oh = np.zeros((NCORES, GP, 128, TT * 128), BF16)
    oh[sc, sg, q, tile_of * 128 + sd] = snorm.astype(BF16)

    # ---- build + run the SPMD graph ----
    nc = _build_graph(
        T0, T1, [int(v) for v in t0s], [int(v) for v in t1s],
        debug=bool(os.environ.get("A3_DEBUG")),
    )

    wo = np.zeros((128, 16), np.float32)
    wo[:, :T] = W_out.T
    bo = np.zeros((16, 1), np.float32)
    bo[:T, 0] = b_out
    uza = np.zeros((128, 128), np.float32)
    uzb = np.zeros((128, 128), np.float32)
    uha = np.zeros((128, 128), np.float32)
    uhb = np.zeros((128, 128), np.float32)
    for j in range(4):
        uza[32 * j : 32 * j + 16] = Uz
        uzb[32 * j + 16 : 32 * j + 32] = Uz
        uha[32 * j : 32 * j + 16] = Uh
        uhb[32 * j + 16 : 32 * j + 32] = Uh
    uza = uza.astype(BF16)
    uzb = uzb.astype(BF16)
    uha = uha.astype(BF16)
    uhb = uhb.astype(BF16)
    czc = np.ascontiguousarray(cz.reshape(128, 1))
    chc = np.ascontiguousarray(ch.reshape(128, 1))
    wobf = wo.astype(BF16)
    prt = np.ascontiguousarray(np.tile(probs, (128, 1)).astype(np.float32))
    ident = np.eye(128, dtype=BF16)
    xlo_a = np.ascontiguousarray(Xbf[:HALF])
    xhi_a = np.ascontiguousarray(Xbf[HALF:])
    xown = np.zeros((NCORES, GP * 128, 256), BF16)
    xown[:, :NPC] = Xbf.reshape(NCORES, NPC, 256)
    d2 = np.zeros((NCORES, GP * 128), np.float32)
    d2[:, :NPC] = (1.0 / deg).reshape(NCORES, NPC)
    d2 = np.ascontiguousarray(
        d2.reshape(NCORES, GP, 128).transpose(0, 2, 1)
    )  # [NCORES, 128, GP]

    in_maps = []
    for k in range(NCORES):
        in_maps.append(
            {
                "xlo": xlo_a,
                "xhi": xhi_a,
                "ilo": idx_lo[k],
                "ihi": idx_hi[k],
                "oh": oh[k],
                "uza": uza,
                "uzb": uzb,
                "uha": uha,
                "uhb": uhb,
                "cz": czc,
                "ch": chc,
                "wo": wobf,
                "bo": bo,
                "pr": prt,
                "ident": ident,
                "xown": xown[k],
                "d2": d2[k],
            }
        )

    LAST = _run(nc, in_maps, trace=bool(os.environ.get("KBENCH_TRACE")))

    full = np.zeros((N, T), np.float32)
    for k in range(NCORES):
        full[k * NPC : (k + 1) * NPC, :] = LAST["results"][k]["out"][:T, :NPC].T
    return full


def _ntff_hook():
    """Contextmanager (dir, device_ids) that captures NTFF profiles via the
    axon PJRT .so, replicating the missing antenv.axon_hooks plumbing."""
    import contextlib
    import ctypes

    so_path = "/opt/axon/libaxon_pjrt.so"
    lib = ctypes.CDLL(so_path)
    if not hasattr(lib, "axon_start_nrt_profile"):
        return None
    lib.axon_start_nrt_profile.argtypes = [
        ctypes.POINTER(ctypes.c_int64),
        ctypes.c_size_t,
    ]
    lib.axon_start_nrt_profile.restype = ctypes.c_int64
    lib.axon_stop_nrt_profile.argtypes = [ctypes.c_char_p]
    lib.axon_stop_nrt_profile.restype = ctypes.c_int64

    @contextlib.contextmanager
    def _hook(output_dir, device_ids):
        import jax

        jax.devices()
        if device_ids:
            ids = (ctypes.c_int64 * len(device_ids))(*device_ids)
            rc = lib.axon_start_nrt_profile(ids, len(device_ids))
        else:
            rc = lib.axon_start_nrt_profile(None, 0)
        if rc != 0:
            raise RuntimeError(f"axon_start_nrt_profile rc={rc}")
        try:
            yield
        finally:
            n = lib.axon_stop_nrt_profile(str(output_dir).encode())
            print(f"ntff profile: {n} file(s) -> {output_dir}")

    return _hook


def _run(nc, in_maps, trace=False):
    import tempfile

    from concourse import bass2jax

    out = {"results": None, "exec_time_ns": None, "trace_path": None}
    if not trace:
        out["results"] = bass2jax.run_bass_via_pjrt(nc, in_maps, n_cores=NCORES)
        return out

    hook = _ntff_hook()
    neff_dir = tempfile.mkdtemp(prefix="a3tgcn_prof_")
    # Single profiled run: re-executing the same NEFF wedges the exec unit
    # (NRT_EXEC_UNIT_UNRECOVERABLE), so no separate warmup.
    with hook(neff_dir, [0]):
        out["results"] = bass2jax.run_bass_via_pjrt(nc, in_maps, n_cores=NCORES)

    try:
        import gauge.profiler as gp
        from concourse._compat import FishPath
        from gauge import trn_perfetto

        prof = gp.Profile(
            profile_path=FishPath(neff_dir),
            kernel_dev_mode=True,
            profile_on_exit=False,
            bass_kernel=nc.m,
            offline_processing=True,
            fname="*_body*",
        )
        prof.convert_ntffs_to_json((0,))
        json_path = prof.json_path(0).path
        insts, trace_path, exec_ns, scopes = trn_perfetto.main(
            json=json_path,
            out_path=os.path.join(neff_dir, "trace.pftrace"),
            kernel_dev_mode=True,
            bass_kernel=nc.m,
        )
        out["exec_time_ns"] = exec_ns
        out["trace_path"] = trace_path
        out["neff_dir"] = neff_dir
        out["scope_times"] = scopes
    except Exception as exc:  # profiling must never break the numerics
        print(f"profiling failed: {exc!r}")
    return out
